# revision 1
# baseline (speedup 1.0000x reference)
"""Dilated multi-head self-attention block (B=4, N=2048, D=1024, H=16,
k=8, dilation=2) on 8 Trainium2 NeuronCores.

Sharding: data-parallel over (batch, sequence-half) -> 8 shards of
(1, 1024, 1024) output rows.  Each core receives a halo-extended,
pre-transposed slice of x plus full weights, and computes its output
rows with zero inter-core communication.

Attention structure: (j - i) % dilation == 0 with |j - i| <= k*dilation
decomposes the sequence into `dilation` parity chains; within a chain
the mask is a plain band of half-width k.  Per (head, parity, 128-query
block) a dense 128x144 score block is computed on the PE; the additive
band mask is pre-injected into PSUM by an identity matmul, so a single
Exp activation does mask + scale + exp + row-sum (accum_out) in one
pass.  Softmax normalization happens in the q-partition layout via
tensor_scalar; A is then PE-transposed for the PV matmul, which
produces the attention output directly in feature-major layout for the
final projection.
"""

import numpy as np
import ml_dtypes

import bass_rust
import concourse.bass as bass
import concourse.mybir as mybir
from concourse.tile import TileContext
from concourse.vector_clock import ScopedClock
from concourse.bass_utils import run_bass_kernel_spmd

# ---------------------------------------------------------------- constants
B, N, D, H = 4, 2048, 1024, 16
DH = D // H            # 64
KK, DIL = 8, 2         # band half-width (in chain coords), dilation
HALO = KK * DIL        # 16 rows of sequence halo per side
INT = N // 2           # 1024 interior rows per core
EXT = INT + 2 * HALO   # 1056
CH_INT = INT // 2      # 512 chain positions per parity (interior)
CH_EXT = CH_INT + KK * 2  # 528 (8 halo each side)
QB = 128               # queries per block
NBLK = CH_INT // QB    # 4 blocks per parity chain
KW = QB + 2 * KK       # 144-wide key window per block
NEG = -30000.0         # additive mask value (exp underflows to 0)
NCORES = 8
WSLOTS = 12            # rotating SBUF slots for streamed weight chunks

F32 = mybir.dt.float32
F32R = mybir.dt.float32r
BF16 = mybir.dt.bfloat16

# dtype knobs
PROJ_F32R = True       # run the 4 projections as float32r (full PE rate)
QK_DT = BF16           # dtype of Q/K tiles + mask-inject operands
PV_DT = BF16           # dtype of V / A / A^T tiles (PV matmuls)

_NP = {BF16: ml_dtypes.bfloat16, F32: np.float32}

LAST_RUN_WALL_S = None


def _drain_patch(self, tick_clock, wait_clock):
    """TileContext exit drain carries one sem-wait per instruction.

    The walrus in this container rejects a Drain with >1 sync wait
    ("Too many sync wait commands"), so split the global-clock waits
    onto single-wait SP nops before the drain."""
    nop0 = self.nc.sync.nop(nofuse=True)
    wait_clock.add_sem_waits(nop0.ins, ScopedClock({None: tick_clock.global_clock}))
    si = nop0.ins.sync_info
    waits = list(si.on_wait or []) if si is not None else []
    if len(waits) > 1:
        nop0.ins.sync_info = bass_rust.SyncInfo(
            on_wait=[waits[0]], on_update=list(si.on_update or [])
        )
        for w in waits[1:]:
            n2 = self.nc.sync.nop(nofuse=True)
            n2.ins.sync_info = bass_rust.SyncInfo(on_wait=[w], on_update=[])
    self.nc.sync.drain()
    self.nc.all_engine_barrier()
    popped = self.nc._tile_sem_poison_stack.pop()
    assert popped is self._sem_poison
    self.nc.clear_and_free_semaphores(list(self.sems.allocated().values()))
    self.nc.all_engine_barrier()


def _cast(ap, dt):
    return ap.bitcast(dt) if ap.dtype != dt else ap


_wait_split_installed = [False]


def _install_bir_wait_split():
    """The walrus in this container accepts at most ONE sync wait per
    instruction ("Too many sync wait commands").  Tile's scheduler freely
    emits several.  Rewrite the BIR JSON just before neuronxcc: any
    instruction with N>1 waits gets N-1 single-wait NoOps (same engine)
    inserted right before it — same semantics, engine program order
    preserved."""
    if _wait_split_installed[0]:
        return
    import json
    import concourse.bass2jax as b2j

    orig = b2j.compile_bir_kernel

    def patched(bir_json, tmpdir, neff_name="file.neff"):
        js = json.loads(bir_json)
        n_split = 0
        for fn in js.get("functions", []):
            for bb in fn.get("blocks", []):
                new_insts = []
                for inst in bb.get("instructions", []):
                    si = inst.get("sync_info")
                    ow = (si or {}).get("on_wait") or []
                    if len(ow) > 1:
                        for wi, w in enumerate(ow[:-1]):
                            new_insts.append({
                                "debug": inst.get("debug", 0),
                                "engine": inst["engine"],
                                "ins": [], "outs": [],
                                "name": f"{inst['name']}_wsplit{wi}",
                                "opcode": "NoOp",
                                "sync_info": {"on_update": [], "on_wait": [w]},
                            })
                            n_split += 1
                        si["on_wait"] = [ow[-1]]
                    new_insts.append(inst)
                bb["instructions"] = new_insts
        return orig(json.dumps(js).encode(), tmpdir, neff_name)

    b2j.compile_bir_kernel = patched
    _wait_split_installed[0] = True


def build_program(with_biases, phases="all"):
    """One SPMD program; per-core differences come in through the inputs."""
    nc = bass.Bass("TRN2", target_bir_lowering=False, debug=False,
                   num_devices=NCORES)
    AF = mybir.ActivationFunctionType

    pdt = F32R if PROJ_F32R else F32
    xT_d = nc.dram_tensor("xT", [D, EXT], pdt, kind="ExternalInput").ap()
    w_d = {p: nc.dram_tensor(f"W{p}", [D, D], pdt, kind="ExternalInput").ap()
           for p in "qkvo"}
    bqk_d = nc.dram_tensor("bqk", [D, 2], F32, kind="ExternalInput").ap()
    bvo_d = (nc.dram_tensor("bvo", [2, D], F32, kind="ExternalInput").ap()
             if with_biases else None)
    m_d = [nc.dram_tensor(f"mask{i}", [QB, KW], QK_DT, kind="ExternalInput").ap()
           for i in range(3)]
    idqk_d = nc.dram_tensor("idqk", [128, 128], QK_DT, kind="ExternalInput").ap()
    idpv_d = (idqk_d if PV_DT == QK_DT else
              nc.dram_tensor("idpv", [128, 128], PV_DT, kind="ExternalInput").ap())
    # ONLY declare params that are actually read: an unused ExternalInput
    # gets dropped from the NEFF and the PJRT call then fails with an
    # input-count mismatch (INVALID_ARGUMENT).
    ones_d = (nc.dram_tensor("onesrow", [1, 128], PV_DT, kind="ExternalInput").ap()
              if with_biases else None)
    out_d = nc.dram_tensor("out", [INT, D], F32, kind="ExternalOutput").ap()

    with TileContext(nc) as tc:
        # All pools persist for the whole program: mid-context pool release
        # reuses memory without cross-pool synchronization (CoreSim flags
        # the race), so everything lives side by side instead.
        with tc.tile_pool(name="const", bufs=1) as cpool, \
             tc.tile_pool(name="wpool", bufs=1) as wpool, \
             tc.tile_pool(name="qkpool", bufs=1) as qkpool, \
             tc.tile_pool(name="vpool", bufs=1) as vpool, \
             tc.tile_pool(name="xpool", bufs=1) as xpool, \
             tc.tile_pool(name="otpool", bufs=1) as otpool, \
             tc.tile_pool(name="apool", bufs=2) as apool, \
             tc.tile_pool(name="atpool", bufs=3) as atpool, \
             tc.tile_pool(name="smpool", bufs=3) as smpool, \
             tc.tile_pool(name="outpool", bufs=2) as outpool, \
             tc.tile_pool(name="ppsum", bufs=2, space="PSUM") as ppsum, \
             tc.tile_pool(name="spsum", bufs=2, space="PSUM") as spsum, \
             tc.tile_pool(name="atpsum", bufs=2, space="PSUM") as atpsum, \
             tc.tile_pool(name="pvpsum", bufs=2, space="PSUM") as pvpsum:

            # ------------------------------------------------ constants
            masks = []
            for i in range(3):
                mt = cpool.tile([QB, KW], QK_DT, tag=f"mask{i}", name=f"mask{i}_sb")
                nc.sync.dma_start(out=mt, in_=m_d[i])
                masks.append(mt)
            idqk = cpool.tile([128, 128], QK_DT, tag="idqk", name="idqk_sb")
            nc.sync.dma_start(out=idqk, in_=idqk_d)
            if PV_DT == QK_DT:
                idpv = idqk
            else:
                idpv = cpool.tile([128, 128], PV_DT, tag="idpv", name="idpv_sb")
                nc.sync.dma_start(out=idpv, in_=idpv_d)
            bqk = cpool.tile([128, 8, 2], F32, tag="bqk", name="bqk_sb")
            nc.sync.dma_start(out=bqk, in_=bqk_d.rearrange("(m p) t -> p m t", p=128))
            if with_biases:
                bvo = cpool.tile([1, 2, D], PV_DT, tag="bvo", name="bvo_sb")
                nc.sync.dma_start(out=bvo, in_=bvo_d.rearrange("t d -> 1 t d"))
                onesrow = cpool.tile([1, 128], PV_DT, tag="ones", name="ones_sb")
                nc.sync.dma_start(out=onesrow, in_=ones_d)

            # ------------------------------------------------ persistent arrays
            QT = [qkpool.tile([128, INT], QK_DT, tag=f"qt{m}", name=f"qt{m}")
                  for m in range(8)]
            KT = [qkpool.tile([128, EXT], QK_DT, tag=f"kt{m}", name=f"kt{m}")
                  for m in range(8)]
            # V in natural layout, de-interleaved per parity; 4 full chunks
            # of 128 chain rows + one 16-row tail per parity
            VCH = [128, 128, 128, 128, 16]
            V = [[vpool.tile([VCH[v], D], PV_DT, tag=f"v{p}_{v}", name=f"v{p}_{v}")
                  for v in range(5)] for p in range(2)]
            OT = [otpool.tile([128, INT], pdt, tag=f"ot{m}", name=f"ot{m}")
                  for m in range(8)]

            xT = []
            for k in range(8):
                xt = xpool.tile([128, EXT], pdt, tag=f"xt{k}", name=f"xt{k}")
                nc.sync.dma_start(out=xt, in_=xT_d[k * 128:(k + 1) * 128, :])
                xT.append(xt)
            xTr = [t.rearrange("d (c two) -> d c two", two=2) for t in xT]

            # weight chunks stream through WSLOTS rotating single-buffer
            # slots so the next projection's chunks prefetch while the
            # current projection still holds its own
            wslot = [0]

            def load_w(which):
                tiles = []
                for k in range(8):
                    slot = (wslot[0] + k) % WSLOTS
                    wt = wpool.tile([128, D], pdt, tag=f"w{slot}",
                                    name=f"w_{which}{k}")
                    nc.sync.dma_start(out=wt, in_=w_d[which][k * 128:(k + 1) * 128, :])
                    tiles.append(wt)
                wslot[0] = (wslot[0] + 8) % WSLOTS
                return tiles

            # ------------------------------------------------ projections
            # V projection: out V[p][v][rows, dout], lhsT = xT parity slice
            wv = load_w("v")
            for p in range(2):
                for v in range(5):
                    rows = VCH[v]
                    for n in range(2):
                        ps = ppsum.tile([128, 512], F32, tag="ppsum", name="psV")
                        for k in range(8):
                            nc.tensor.matmul(
                                ps[:rows, :],
                                lhsT=xTr[k][:, v * 128:v * 128 + rows, p],
                                rhs=wv[k][:, n * 512:(n + 1) * 512],
                                start=(k == 0), stop=(k == 7 and not with_biases))
                        if with_biases:
                            nc.tensor.matmul(
                                ps[:rows, :], lhsT=onesrow[:, :rows],
                                rhs=bvo[0:1, 0, n * 512:(n + 1) * 512],
                                start=False, stop=True)
                        eng = (v + n) % 2
                        if eng:
                            nc.scalar.copy(V[p][v][:rows, n * 512:(n + 1) * 512],
                                           ps[:rows, :])
                        else:
                            nc.vector.tensor_copy(V[p][v][:rows, n * 512:(n + 1) * 512],
                                                  ps[:rows, :])

            # Q/K projections: out (Q or K)^T [dout, seq]
            for which, dst, chunks, off, bcol in (
                    ("q", QT, [(0, 512), (512, 512)], HALO, 0),
                    ("k", KT, [(0, 512), (512, 512), (1024, 32)], 0, 1)):
                wt = load_w(which)
                for m in range(8):
                    for (s0, sl) in chunks:
                        ps = ppsum.tile([128, 512], F32, tag="ppsum", name="psQK")
                        for k in range(8):
                            nc.tensor.matmul(
                                ps[:, :sl],
                                lhsT=wt[k][:, m * 128:(m + 1) * 128],
                                rhs=xT[k][:, off + s0: off + s0 + sl],
                                start=(k == 0), stop=(k == 7))
                        nc.scalar.activation(
                            dst[m][:, s0:s0 + sl], ps[:, :sl], AF.Identity,
                            bias=bqk[:, m, bcol:bcol + 1])

            wo = load_w("o")

            if phases == "proj":
                # debug: skip attention/O-proj; dump V through out
                for s in range(8):
                    ot_out = outpool.tile([128, D], F32, tag="out", name="dbg_out")
                    nc.vector.tensor_copy(ot_out, V[s % 2][s % 4])
                    nc.sync.dma_start(out=out_d[s * 128:(s + 1) * 128, :], in_=ot_out)
                return_early = True
            else:
                return_early = False

            # ------------------------------------------------ attention
            OTr = [t.rearrange("d (c two) -> d c two", two=2) for t in OT]
            QTr = [t.rearrange("d (c two) -> d c two", two=2) for t in QT]
            KTr = [t.rearrange("d (c two) -> d c two", two=2) for t in KT]

            for b in range(0 if return_early else NBLK):
                for p in range(2):
                    mt = masks[0] if b == 0 else (masks[2] if b == NBLK - 1 else masks[1])
                    sums = smpool.tile([128, 16], F32, tag="sums", name="sums")
                    A = apool.tile([128, 16, KW], PV_DT, tag="A", name="Atile")
                    for h in range(16):
                        mch, mrow = h // 2, (h % 2) * 64
                        sps = spsum.tile([QB, KW], F32, tag="s", name="spsum")
                        nc.tensor.matmul(sps, lhsT=idqk, rhs=mt,
                                         start=True, stop=False)
                        nc.tensor.matmul(
                            sps,
                            lhsT=QTr[mch][mrow:mrow + 64, b * QB:(b + 1) * QB, p],
                            rhs=KTr[mch][mrow:mrow + 64, b * QB:b * QB + KW, p],
                            start=False, stop=True)
                        nc.scalar.activation(
                            A[:, h, :], sps, AF.Exp, scale=0.125,
                            accum_out=sums[:, h:h + 1])
                    rec = smpool.tile([128, 16], F32, tag="rec", name="rec")
                    nc.vector.reciprocal(rec, sums)
                    for h in range(16):
                        mch, mrow = h // 2, (h % 2) * 64
                        nc.vector.tensor_scalar_mul(
                            A[:, h, :], A[:, h, :], rec[:, h:h + 1])
                        atp = atpsum.tile([128, 256], PV_DT, tag="at", name="atpsum")
                        nc.tensor.transpose(atp[:, 0:128], A[:, h, 0:QB], idpv)
                        nc.tensor.transpose(atp[0:2 * KK, 128:256],
                                            A[:, h, QB:KW], idpv)
                        at = atpool.tile([128, 256], PV_DT, tag="at", name="at_sb")
                        if h % 2:
                            nc.scalar.copy(at[:, 0:128], atp[:, 0:128])
                            nc.scalar.copy(at[0:2 * KK, 128:256],
                                           atp[0:2 * KK, 128:256])
                        else:
                            nc.vector.tensor_copy(at[:, 0:128], atp[:, 0:128])
                            nc.vector.tensor_copy(at[0:2 * KK, 128:256],
                                                  atp[0:2 * KK, 128:256])
                        pvp = pvpsum.tile([64, 128], F32, tag="pv", name="pvpsum")
                        nc.tensor.matmul(pvp, lhsT=V[p][b][:, h * DH:(h + 1) * DH],
                                         rhs=at[:, 0:128], start=True, stop=False)
                        nc.tensor.matmul(pvp,
                                         lhsT=V[p][b + 1][0:2 * KK, h * DH:(h + 1) * DH],
                                         rhs=at[0:2 * KK, 128:256],
                                         start=False, stop=True)
                        dst = OTr[mch][mrow:mrow + 64, b * QB:(b + 1) * QB, p]
                        if h % 2:
                            nc.vector.tensor_copy(dst, pvp)
                        else:
                            nc.scalar.copy(dst, pvp)

                # ---------------------------------- output projection for the
                # two interior seq chunks completed by this block
                for s in (2 * b, 2 * b + 1):
                    ot_out = outpool.tile([128, D], F32, tag="out", name="out_sb")
                    for n in range(2):
                        ps = ppsum.tile([128, 512], F32, tag="ppsum", name="opsum")
                        for k in range(8):
                            nc.tensor.matmul(
                                ps,
                                lhsT=OT[k][:, s * 128:(s + 1) * 128],
                                rhs=wo[k][:, n * 512:(n + 1) * 512],
                                start=(k == 0), stop=(k == 7 and not with_biases))
                        if with_biases:
                            nc.tensor.matmul(
                                ps, lhsT=onesrow,
                                rhs=bvo[0:1, 1, n * 512:(n + 1) * 512],
                                start=False, stop=True)
                        if n:
                            nc.scalar.copy(ot_out[:, n * 512:(n + 1) * 512], ps)
                        else:
                            nc.vector.tensor_copy(ot_out[:, n * 512:(n + 1) * 512], ps)
                    nc.sync.dma_start(out=out_d[s * 128:(s + 1) * 128, :], in_=ot_out)
    return nc


def _host_inputs(x, Wq, bq, Wk, bk, Wv, bv, Wo, bo):
    """Build the 8 per-core input maps."""
    qknp = _NP[QK_DT]
    pvnp = _NP[PV_DT]

    # band masks in block-local chain coords: allowed iff 0 <= j - i <= 16
    i = np.arange(QB)[:, None]
    j = np.arange(KW)[None, :]
    band = (j - i >= 0) & (j - i <= 2 * KK)
    m_mid = np.where(band, 0.0, NEG).astype(np.float32)
    # halo is KK chain positions wide; clip keys that fall outside [0, N)
    m_left = np.where(band & (j >= KK), 0.0, NEG).astype(np.float32)
    m_right = np.where(band & (j < KW - KK), 0.0, NEG).astype(np.float32)

    ident = np.eye(128, dtype=np.float32)
    bqk = np.stack([bq, bk], axis=1).astype(np.float32)          # [D, 2]
    bvo = np.stack([bv, bo], axis=0).astype(np.float32)          # [2, D]
    onesrow = np.ones((1, 128), dtype=np.float32)

    xp = np.zeros((B, N + 2 * HALO, D), dtype=np.float32)
    xp[:, HALO:HALO + N] = x

    shared = {
        "Wq": np.ascontiguousarray(Wq, np.float32),
        "Wk": np.ascontiguousarray(Wk, np.float32),
        "Wv": np.ascontiguousarray(Wv, np.float32),
        "Wo": np.ascontiguousarray(Wo, np.float32),
        "bqk": bqk, "bvo": bvo.astype(pvnp),
        "mask1": m_mid.astype(qknp),
        "idqk": ident.astype(qknp),
        "onesrow": onesrow.astype(pvnp),
    }
    if PV_DT != QK_DT:
        shared["idpv"] = ident.astype(pvnp)

    with_biases = bool(np.any(bv) or np.any(bo))
    if not with_biases:
        shared.pop("bvo", None)
        shared.pop("onesrow", None)

    in_maps = []
    for core in range(NCORES):
        bi, half = core // 2, core % 2
        xT = np.ascontiguousarray(
            xp[bi, half * INT: half * INT + EXT].T, np.float32)
        m0 = m_left if half == 0 else m_mid
        m2 = m_right if half == 1 else m_mid
        im = dict(shared)
        im["xT"] = xT
        im["mask0"] = m0.astype(qknp)
        im["mask2"] = m2.astype(qknp)
        in_maps.append(im)
    return in_maps


_prog_cache = {}


def kernel(x, Wq, bq, Wk, bk, Wv, bv, Wo, bo, k, dilation, **_unused):
    x = np.asarray(x, np.float32)
    assert int(k) == KK and int(dilation) == DIL, (k, dilation)
    assert x.shape == (B, N, D)
    bq, bk, bv, bo = (np.asarray(v, np.float32).reshape(D) for v in (bq, bk, bv, bo))

    TileContext._drain_and_barrier = _drain_patch
    _install_bir_wait_split()

    with_biases = bool(np.any(bv) or np.any(bo))
    key = with_biases
    if key not in _prog_cache:
        _prog_cache[key] = build_program(with_biases)
    nc = _prog_cache[key]

    in_maps = _host_inputs(x, np.asarray(Wq), bq, np.asarray(Wk), bk,
                           np.asarray(Wv), bv, np.asarray(Wo), bo)
    import time as _time
    _t0 = _time.monotonic()
    res = run_bass_kernel_spmd(nc, in_maps, list(range(NCORES)))
    global LAST_RUN_WALL_S
    LAST_RUN_WALL_S = _time.monotonic() - _t0

    out = np.empty((B, N, D), dtype=np.float32)
    for core in range(NCORES):
        bi, half = core // 2, core % 2
        out[bi, half * INT:(half + 1) * INT] = res.results[core]["out"]
    return out



# revision 9
# speedup vs baseline: 3.7780x; 3.7780x over previous
"""Dilated multi-head self-attention block (B=4, N=2048, D=1024, H=16,
k=8, dilation=2) on 8 Trainium2 NeuronCores.

Sharding: data-parallel over (batch, sequence-half) -> 8 shards of
(1, 1024, 1024) output rows.  Each core receives a halo-extended,
pre-transposed slice of x plus full weights, and computes its output
rows with zero inter-core communication.

Attention structure: (j - i) % dilation == 0 with |j - i| <= k*dilation
decomposes the sequence into `dilation` parity chains; within a chain
the mask is a plain band of half-width k.  Per (head, parity, 128-query
block) a dense 128x144 score block is computed on the PE; the additive
band mask is pre-injected into PSUM by an identity matmul, so a single
Exp activation does mask + scale + exp + row-sum (accum_out) in one
pass.  Softmax normalization happens in the q-partition layout via
tensor_scalar; A is then PE-transposed for the PV matmul, which
produces the attention output directly in feature-major layout for the
final projection.
"""

import numpy as np
import ml_dtypes

import bass_rust
import concourse.bass as bass
import concourse.mybir as mybir
from concourse.tile import TileContext
from concourse.vector_clock import ScopedClock
from concourse.bass_utils import run_bass_kernel_spmd

# ---------------------------------------------------------------- constants
B, N, D, H = 4, 2048, 1024, 16
DH = D // H            # 64
KK, DIL = 8, 2         # band half-width (in chain coords), dilation
HALO = KK * DIL        # 16 rows of sequence halo per side
INT = N // 2           # 1024 interior rows per core
EXT = INT + 2 * HALO   # 1056
CH_INT = INT // 2      # 512 chain positions per parity (interior)
CH_EXT = CH_INT + KK * 2  # 528 (8 halo each side)
QB = 128               # queries per block
NBLK = CH_INT // QB    # 4 blocks per parity chain
KW = QB + 2 * KK       # 144-wide key window per block
NEG = -30000.0         # additive mask value (exp underflows to 0)
NCORES = 8
WSLOTS = 12            # rotating SBUF slots for streamed weight chunks

F32 = mybir.dt.float32
F32R = mybir.dt.float32r
BF16 = mybir.dt.bfloat16
F16 = mybir.dt.float16

# dtype knobs.  Everything 16-bit: fp16 runs at full PE rate (same as
# bf16) with 3 more mantissa bits, and halves every tunnel transfer +
# HBM/SBUF byte moved vs fp32.
PROJ_DT = F16          # x / W / attention-out tiles (all 4 projections)
QK_DT = F16            # dtype of Q/K tiles + mask-inject operands
PV_DT = F16            # dtype of V / A / A^T tiles (PV matmuls)
OUT_DT = F16           # DRAM output dtype (host upcasts to fp32)

_NP = {BF16: ml_dtypes.bfloat16, F16: np.float16, F32: np.float32}

LAST_RUN_WALL_S = None


def _drain_patch(self, tick_clock, wait_clock):
    """TileContext exit drain carries one sem-wait per instruction.

    The walrus in this container rejects a Drain with >1 sync wait
    ("Too many sync wait commands"), so split the global-clock waits
    onto single-wait SP nops before the drain."""
    nop0 = self.nc.sync.nop(nofuse=True)
    wait_clock.add_sem_waits(nop0.ins, ScopedClock({None: tick_clock.global_clock}))
    si = nop0.ins.sync_info
    waits = list(si.on_wait or []) if si is not None else []
    if len(waits) > 1:
        nop0.ins.sync_info = bass_rust.SyncInfo(
            on_wait=[waits[0]], on_update=list(si.on_update or [])
        )
        for w in waits[1:]:
            n2 = self.nc.sync.nop(nofuse=True)
            n2.ins.sync_info = bass_rust.SyncInfo(on_wait=[w], on_update=[])
    self.nc.sync.drain()
    self.nc.all_engine_barrier()
    popped = self.nc._tile_sem_poison_stack.pop()
    assert popped is self._sem_poison
    self.nc.clear_and_free_semaphores(list(self.sems.allocated().values()))
    self.nc.all_engine_barrier()


def _cast(ap, dt):
    return ap.bitcast(dt) if ap.dtype != dt else ap


_wait_split_installed = [False]


def _install_bir_wait_split():
    """The walrus in this container accepts at most ONE sync wait per
    instruction ("Too many sync wait commands").  Tile's scheduler freely
    emits several.  Rewrite the BIR JSON just before neuronxcc: any
    instruction with N>1 waits gets N-1 single-wait NoOps (same engine)
    inserted right before it — same semantics, engine program order
    preserved."""
    if _wait_split_installed[0]:
        return
    import json
    import concourse.bass2jax as b2j

    orig = b2j.compile_bir_kernel

    def patched(bir_json, tmpdir, neff_name="file.neff"):
        js = json.loads(bir_json)
        n_split = 0
        for fn in js.get("functions", []):
            for bb in fn.get("blocks", []):
                new_insts = []
                for inst in bb.get("instructions", []):
                    si = inst.get("sync_info")
                    ow = (si or {}).get("on_wait") or []
                    if len(ow) > 1:
                        for wi, w in enumerate(ow[:-1]):
                            new_insts.append({
                                "debug": inst.get("debug", 0),
                                "engine": inst["engine"],
                                "ins": [], "outs": [],
                                "name": f"{inst['name']}_wsplit{wi}",
                                "opcode": "NoOp",
                                "sync_info": {"on_update": [], "on_wait": [w]},
                            })
                            n_split += 1
                        si["on_wait"] = [ow[-1]]
                    new_insts.append(inst)
                bb["instructions"] = new_insts
        return orig(json.dumps(js).encode(), tmpdir, neff_name)

    b2j.compile_bir_kernel = patched
    _wait_split_installed[0] = True


def build_program(with_biases, phases="all"):
    """One SPMD program; per-core differences come in through the inputs."""
    nc = bass.Bass("TRN2", target_bir_lowering=False, debug=False,
                   num_devices=NCORES)
    AF = mybir.ActivationFunctionType

    pdt = PROJ_DT
    xT_d = nc.dram_tensor("xT", [D, EXT], pdt, kind="ExternalInput").ap()
    # Each core ships only ITS 128-row shard of every weight matrix; the
    # full matrices are rebuilt on-device with an AllGather over the 8
    # cores (NeuronLink), cutting host->device weight bytes 8x.
    wsh_d = {p: nc.dram_tensor(f"W{p}s", [128, D], pdt, kind="ExternalInput").ap()
             for p in "qkvo"}
    bqk_d = nc.dram_tensor("bqk", [D, 2], F32, kind="ExternalInput").ap()
    bvo_d = (nc.dram_tensor("bvo", [2, D], F32, kind="ExternalInput").ap()
             if with_biases else None)
    m_d = [nc.dram_tensor(f"mask{i}", [QB, KW], QK_DT, kind="ExternalInput").ap()
           for i in range(3)]
    idqk_d = nc.dram_tensor("idqk", [128, 128], QK_DT, kind="ExternalInput").ap()
    idpv_d = (idqk_d if PV_DT == QK_DT else
              nc.dram_tensor("idpv", [128, 128], PV_DT, kind="ExternalInput").ap())
    # ONLY declare params that are actually read: an unused ExternalInput
    # gets dropped from the NEFF and the PJRT call then fails with an
    # input-count mismatch (INVALID_ARGUMENT).
    ones_d = (nc.dram_tensor("onesrow", [1, 128], PV_DT, kind="ExternalInput").ap()
              if with_biases else None)
    out_d = nc.dram_tensor("out", [INT, D], OUT_DT, kind="ExternalOutput").ap()

    with TileContext(nc) as tc:
        # All pools persist for the whole program: mid-context pool release
        # reuses memory without cross-pool synchronization (CoreSim flags
        # the race), so everything lives side by side instead.
        with tc.tile_pool(name="dram", bufs=1, space="DRAM") as dpool, \
             tc.tile_pool(name="const", bufs=1) as cpool, \
             tc.tile_pool(name="wpool", bufs=1) as wpool, \
             tc.tile_pool(name="qkpool", bufs=1) as qkpool, \
             tc.tile_pool(name="vpool", bufs=1) as vpool, \
             tc.tile_pool(name="xpool", bufs=1) as xpool, \
             tc.tile_pool(name="otpool", bufs=1) as otpool, \
             tc.tile_pool(name="apool", bufs=2) as apool, \
             tc.tile_pool(name="atpool", bufs=3) as atpool, \
             tc.tile_pool(name="smpool", bufs=3) as smpool, \
             tc.tile_pool(name="outpool", bufs=2) as outpool, \
             tc.tile_pool(name="ppsum", bufs=2, space="PSUM") as ppsum, \
             tc.tile_pool(name="spsum", bufs=2, space="PSUM") as spsum, \
             tc.tile_pool(name="atpsum", bufs=2, space="PSUM") as atpsum, \
             tc.tile_pool(name="pvpsum", bufs=2, space="PSUM") as pvpsum:

            # ---------------------------------------- weight all-gather
            # Issue the 4 collectives first, in consumption order, so the
            # wire time overlaps the x/mask/constant DMAs and each other.
            wgath = {}
            for p in "vqko":
                wb = dpool.tile([128, D], pdt, tag=f"wb{p}", name=f"wb{p}")
                nc.gpsimd.dma_start(wb[:], wsh_d[p])
                gg = dpool.tile([NCORES, 128, D], pdt, tag=f"wg{p}",
                                name=f"wg{p}")
                nc.gpsimd.collective_compute(
                    "AllGather", mybir.AluOpType.bypass,
                    replica_groups=[list(range(NCORES))],
                    ins=[wb.opt()], outs=[gg.opt()])
                wgath[p] = gg

            # ------------------------------------------------ constants
            masks = []
            for i in range(3):
                mt = cpool.tile([QB, KW], QK_DT, tag=f"mask{i}", name=f"mask{i}_sb")
                nc.sync.dma_start(out=mt, in_=m_d[i])
                masks.append(mt)
            idqk = cpool.tile([128, 128], QK_DT, tag="idqk", name="idqk_sb")
            nc.sync.dma_start(out=idqk, in_=idqk_d)
            if PV_DT == QK_DT:
                idpv = idqk
            else:
                idpv = cpool.tile([128, 128], PV_DT, tag="idpv", name="idpv_sb")
                nc.sync.dma_start(out=idpv, in_=idpv_d)
            bqk = cpool.tile([128, 8, 2], F32, tag="bqk", name="bqk_sb")
            nc.sync.dma_start(out=bqk, in_=bqk_d.rearrange("(m p) t -> p m t", p=128))
            if with_biases:
                bvo = cpool.tile([1, 2, D], PV_DT, tag="bvo", name="bvo_sb")
                nc.sync.dma_start(out=bvo, in_=bvo_d.rearrange("t d -> 1 t d"))
                onesrow = cpool.tile([1, 128], PV_DT, tag="ones", name="ones_sb")
                nc.sync.dma_start(out=onesrow, in_=ones_d)

            # ------------------------------------------------ persistent arrays
            QT = [qkpool.tile([128, INT], QK_DT, tag=f"qt{m}", name=f"qt{m}")
                  for m in range(8)]
            KT = [qkpool.tile([128, EXT], QK_DT, tag=f"kt{m}", name=f"kt{m}")
                  for m in range(8)]
            # V in natural layout, de-interleaved per parity; 4 full chunks
            # of 128 chain rows + one 16-row tail per parity
            VCH = [128, 128, 128, 128, 16]
            V = [[vpool.tile([VCH[v], D], PV_DT, tag=f"v{p}_{v}", name=f"v{p}_{v}")
                  for v in range(5)] for p in range(2)]
            OT = [otpool.tile([128, INT], pdt, tag=f"ot{m}", name=f"ot{m}")
                  for m in range(8)]

            xT = []
            for k in range(8):
                xt = xpool.tile([128, EXT], pdt, tag=f"xt{k}", name=f"xt{k}")
                nc.sync.dma_start(out=xt, in_=xT_d[k * 128:(k + 1) * 128, :])
                xT.append(xt)
            xTr = [t.rearrange("d (c two) -> d c two", two=2) for t in xT]

            # weight chunks stream through WSLOTS rotating single-buffer
            # slots so the next projection's chunks prefetch while the
            # current projection still holds its own
            wslot = [0]

            def load_w(which):
                tiles = []
                for k in range(8):
                    slot = (wslot[0] + k) % WSLOTS
                    wt = wpool.tile([128, D], pdt, tag=f"w{slot}",
                                    name=f"w_{which}{k}")
                    nc.sync.dma_start(out=wt, in_=wgath[which][k])
                    tiles.append(wt)
                wslot[0] = (wslot[0] + 8) % WSLOTS
                return tiles

            # ------------------------------------------------ projections
            # V projection: out V[p][v][rows, dout], lhsT = xT parity slice
            wv = load_w("v")
            for p in range(2):
                for v in range(5):
                    rows = VCH[v]
                    for n in range(2):
                        ps = ppsum.tile([128, 512], F32, tag="ppsum", name="psV")
                        for k in range(8):
                            nc.tensor.matmul(
                                ps[:rows, :],
                                lhsT=xTr[k][:, v * 128:v * 128 + rows, p],
                                rhs=wv[k][:, n * 512:(n + 1) * 512],
                                start=(k == 0), stop=(k == 7 and not with_biases))
                        if with_biases:
                            nc.tensor.matmul(
                                ps[:rows, :], lhsT=onesrow[:, :rows],
                                rhs=bvo[0:1, 0, n * 512:(n + 1) * 512],
                                start=False, stop=True)
                        eng = (v + n) % 2
                        if eng:
                            nc.scalar.copy(V[p][v][:rows, n * 512:(n + 1) * 512],
                                           ps[:rows, :])
                        else:
                            nc.vector.tensor_copy(V[p][v][:rows, n * 512:(n + 1) * 512],
                                                  ps[:rows, :])

            # Q/K projections: out (Q or K)^T [dout, seq]
            for which, dst, chunks, off, bcol in (
                    ("q", QT, [(0, 512), (512, 512)], HALO, 0),
                    ("k", KT, [(0, 512), (512, 512), (1024, 32)], 0, 1)):
                wt = load_w(which)
                for m in range(8):
                    for (s0, sl) in chunks:
                        ps = ppsum.tile([128, 512], F32, tag="ppsum", name="psQK")
                        for k in range(8):
                            nc.tensor.matmul(
                                ps[:, :sl],
                                lhsT=wt[k][:, m * 128:(m + 1) * 128],
                                rhs=xT[k][:, off + s0: off + s0 + sl],
                                start=(k == 0), stop=(k == 7))
                        nc.scalar.activation(
                            dst[m][:, s0:s0 + sl], ps[:, :sl], AF.Identity,
                            bias=bqk[:, m, bcol:bcol + 1])

            wo = load_w("o")

            if phases == "proj":
                # debug: skip attention/O-proj; dump V through out
                for s in range(8):
                    ot_out = outpool.tile([128, D], OUT_DT, tag="out", name="dbg_out")
                    nc.vector.tensor_copy(ot_out, V[s % 2][s % 4])
                    nc.sync.dma_start(out=out_d[s * 128:(s + 1) * 128, :], in_=ot_out)
                return_early = True
            else:
                return_early = False

            # ------------------------------------------------ attention
            OTr = [t.rearrange("d (c two) -> d c two", two=2) for t in OT]
            QTr = [t.rearrange("d (c two) -> d c two", two=2) for t in QT]
            KTr = [t.rearrange("d (c two) -> d c two", two=2) for t in KT]

            for b in range(0 if return_early else NBLK):
                for p in range(2):
                    mt = masks[0] if b == 0 else (masks[2] if b == NBLK - 1 else masks[1])
                    sums = smpool.tile([128, 16], F32, tag="sums", name="sums")
                    A = apool.tile([128, 16, KW], PV_DT, tag="A", name="Atile")
                    for h in range(16):
                        mch, mrow = h // 2, (h % 2) * 64
                        sps = spsum.tile([QB, KW], F32, tag="s", name="spsum")
                        nc.tensor.matmul(sps, lhsT=idqk, rhs=mt,
                                         start=True, stop=False)
                        nc.tensor.matmul(
                            sps,
                            lhsT=QTr[mch][mrow:mrow + 64, b * QB:(b + 1) * QB, p],
                            rhs=KTr[mch][mrow:mrow + 64, b * QB:b * QB + KW, p],
                            start=False, stop=True)
                        nc.scalar.activation(
                            A[:, h, :], sps, AF.Exp, scale=0.125,
                            accum_out=sums[:, h:h + 1])
                    rec = smpool.tile([128, 16], F32, tag="rec", name="rec")
                    nc.vector.reciprocal(rec, sums)
                    for h in range(16):
                        mch, mrow = h // 2, (h % 2) * 64
                        nc.vector.tensor_scalar_mul(
                            A[:, h, :], A[:, h, :], rec[:, h:h + 1])
                        atp = atpsum.tile([128, 256], PV_DT, tag="at", name="atpsum")
                        nc.tensor.transpose(atp[:, 0:128], A[:, h, 0:QB], idpv)
                        nc.tensor.transpose(atp[0:2 * KK, 128:256],
                                            A[:, h, QB:KW], idpv)
                        at = atpool.tile([128, 256], PV_DT, tag="at", name="at_sb")
                        if h % 2:
                            nc.scalar.copy(at[:, 0:128], atp[:, 0:128])
                            nc.scalar.copy(at[0:2 * KK, 128:256],
                                           atp[0:2 * KK, 128:256])
                        else:
                            nc.vector.tensor_copy(at[:, 0:128], atp[:, 0:128])
                            nc.vector.tensor_copy(at[0:2 * KK, 128:256],
                                                  atp[0:2 * KK, 128:256])
                        pvp = pvpsum.tile([64, 128], F32, tag="pv", name="pvpsum")
                        nc.tensor.matmul(pvp, lhsT=V[p][b][:, h * DH:(h + 1) * DH],
                                         rhs=at[:, 0:128], start=True, stop=False)
                        nc.tensor.matmul(pvp,
                                         lhsT=V[p][b + 1][0:2 * KK, h * DH:(h + 1) * DH],
                                         rhs=at[0:2 * KK, 128:256],
                                         start=False, stop=True)
                        dst = OTr[mch][mrow:mrow + 64, b * QB:(b + 1) * QB, p]
                        if h % 2:
                            nc.vector.tensor_copy(dst, pvp)
                        else:
                            nc.scalar.copy(dst, pvp)

                # ---------------------------------- output projection for the
                # two interior seq chunks completed by this block
                for s in (2 * b, 2 * b + 1):
                    ot_out = outpool.tile([128, D], OUT_DT, tag="out", name="out_sb")
                    for n in range(2):
                        ps = ppsum.tile([128, 512], F32, tag="ppsum", name="opsum")
                        for k in range(8):
                            nc.tensor.matmul(
                                ps,
                                lhsT=OT[k][:, s * 128:(s + 1) * 128],
                                rhs=wo[k][:, n * 512:(n + 1) * 512],
                                start=(k == 0), stop=(k == 7 and not with_biases))
                        if with_biases:
                            nc.tensor.matmul(
                                ps, lhsT=onesrow,
                                rhs=bvo[0:1, 1, n * 512:(n + 1) * 512],
                                start=False, stop=True)
                        if n:
                            nc.scalar.copy(ot_out[:, n * 512:(n + 1) * 512], ps)
                        else:
                            nc.vector.tensor_copy(ot_out[:, n * 512:(n + 1) * 512], ps)
                    nc.sync.dma_start(out=out_d[s * 128:(s + 1) * 128, :], in_=ot_out)
    return nc


def _host_inputs(x, Wq, bq, Wk, bk, Wv, bv, Wo, bo):
    """Build the 8 per-core input maps."""
    qknp = _NP[QK_DT]
    pvnp = _NP[PV_DT]
    pnp = _NP[PROJ_DT]

    # band masks in block-local chain coords: allowed iff 0 <= j - i <= 16
    i = np.arange(QB)[:, None]
    j = np.arange(KW)[None, :]
    band = (j - i >= 0) & (j - i <= 2 * KK)
    m_mid = np.where(band, 0.0, NEG).astype(np.float32)
    # halo is KK chain positions wide; clip keys that fall outside [0, N)
    m_left = np.where(band & (j >= KK), 0.0, NEG).astype(np.float32)
    m_right = np.where(band & (j < KW - KK), 0.0, NEG).astype(np.float32)

    ident = np.eye(128, dtype=np.float32)
    bqk = np.stack([bq, bk], axis=1).astype(np.float32)          # [D, 2]
    bvo = np.stack([bv, bo], axis=0).astype(np.float32)          # [2, D]
    onesrow = np.ones((1, 128), dtype=np.float32)

    # pad + cast to the 16-bit wire dtype once, then slice per core
    xp = np.zeros((B, N + 2 * HALO, D), dtype=pnp)
    xp[:, HALO:HALO + N] = x

    w16 = {p: np.asarray(w, np.float32).astype(pnp)
           for p, w in (("q", Wq), ("k", Wk), ("v", Wv), ("o", Wo))}

    shared = {
        "bqk": bqk, "bvo": bvo.astype(pvnp),
        "mask1": m_mid.astype(qknp),
        "idqk": ident.astype(qknp),
        "onesrow": onesrow.astype(pvnp),
    }
    if PV_DT != QK_DT:
        shared["idpv"] = ident.astype(pvnp)

    with_biases = bool(np.any(bv) or np.any(bo))
    if not with_biases:
        shared.pop("bvo", None)
        shared.pop("onesrow", None)

    in_maps = []
    for core in range(NCORES):
        bi, half = core // 2, core % 2
        xT = np.ascontiguousarray(xp[bi, half * INT: half * INT + EXT].T)
        m0 = m_left if half == 0 else m_mid
        m2 = m_right if half == 1 else m_mid
        im = dict(shared)
        im["xT"] = xT
        # this core's 128-row shard of each weight matrix (AllGather
        # on-device rebuilds the full [1024, 1024])
        for p in "qkvo":
            im[f"W{p}s"] = np.ascontiguousarray(
                w16[p][core * 128:(core + 1) * 128])
        im["mask0"] = m0.astype(qknp)
        im["mask2"] = m2.astype(qknp)
        in_maps.append(im)
    return in_maps


_prog_cache = {}


def kernel(x, Wq, bq, Wk, bk, Wv, bv, Wo, bo, k, dilation, **_unused):
    x = np.asarray(x, np.float32)
    assert int(k) == KK and int(dilation) == DIL, (k, dilation)
    assert x.shape == (B, N, D)
    bq, bk, bv, bo = (np.asarray(v, np.float32).reshape(D) for v in (bq, bk, bv, bo))

    TileContext._drain_and_barrier = _drain_patch
    _install_bir_wait_split()

    with_biases = bool(np.any(bv) or np.any(bo))
    key = with_biases
    if key not in _prog_cache:
        _prog_cache[key] = build_program(with_biases)
    nc = _prog_cache[key]

    in_maps = _host_inputs(x, np.asarray(Wq), bq, np.asarray(Wk), bk,
                           np.asarray(Wv), bv, np.asarray(Wo), bo)
    import time as _time
    _t0 = _time.monotonic()
    res = run_bass_kernel_spmd(nc, in_maps, list(range(NCORES)))
    global LAST_RUN_WALL_S
    LAST_RUN_WALL_S = _time.monotonic() - _t0

    out = np.empty((B, N, D), dtype=np.float32)
    for core in range(NCORES):
        bi, half = core // 2, core % 2
        out[bi, half * INT:(half + 1) * INT] = res.results[core]["out"]  # f16 -> f32
    return out



# revision 31
# speedup vs baseline: 4.4067x; 1.1664x over previous
"""Dilated multi-head self-attention block (B=4, N=2048, D=1024, H=16,
k=8, dilation=2) on 8 Trainium2 NeuronCores.

Sharding: data-parallel over (batch, sequence-half) -> 8 shards of
(1, 1024, 1024) output rows.  Each core receives a halo-extended,
pre-transposed slice of x plus full weights, and computes its output
rows with zero inter-core communication.

Attention structure: (j - i) % dilation == 0 with |j - i| <= k*dilation
decomposes the sequence into `dilation` parity chains; within a chain
the mask is a plain band of half-width k.  Per (head, parity, 128-query
block) a dense 128x144 score block is computed on the PE; the additive
band mask is pre-injected into PSUM by an identity matmul, so a single
Exp activation does mask + scale + exp + row-sum (accum_out) in one
pass.  Softmax normalization happens in the q-partition layout via
tensor_scalar; A is then PE-transposed for the PV matmul, which
produces the attention output directly in feature-major layout for the
final projection.
"""

import os
import tempfile

import numpy as np
import ml_dtypes

import jax

# Persistent PJRT executable cache: run_bass_kernel_spmd re-jits the
# dispatch wrapper every call; with this cache the per-call XLA compile
# (~0.3s) becomes a disk hit.
jax.config.update("jax_compilation_cache_dir",
                  os.path.join(tempfile.gettempdir(), "jaxcache"))
jax.config.update("jax_persistent_cache_min_entry_size_bytes", 0)
jax.config.update("jax_persistent_cache_min_compile_time_secs", 0.0)

import bass_rust
import concourse.bass as bass
import concourse.mybir as mybir
from concourse.tile import TileContext
from concourse.vector_clock import ScopedClock
from concourse.bass_utils import run_bass_kernel_spmd

# ---------------------------------------------------------------- constants
B, N, D, H = 4, 2048, 1024, 16
DH = D // H            # 64
KK, DIL = 8, 2         # band half-width (in chain coords), dilation
HALO = KK * DIL        # 16 rows of sequence halo per side
INT = N // 2           # 1024 interior rows per core
EXT = INT + 2 * HALO   # 1056
CH_INT = INT // 2      # 512 chain positions per parity (interior)
CH_EXT = CH_INT + KK * 2  # 528 (8 halo each side)
QB = 128               # queries per block
NBLK = CH_INT // QB    # 4 blocks per parity chain
KW = QB + 2 * KK       # 144-wide key window per block
NEG = -30000.0         # additive mask value (exp underflows to 0)
NCORES = 8
WSLOTS = 12            # rotating SBUF slots for streamed weight chunks

F32 = mybir.dt.float32
F32R = mybir.dt.float32r
BF16 = mybir.dt.bfloat16
F16 = mybir.dt.float16

# dtype knobs.  Everything 16-bit: fp16 runs at full PE rate (same as
# bf16) with 3 more mantissa bits, and halves every tunnel transfer +
# HBM/SBUF byte moved vs fp32.
PROJ_DT = F16          # x / W / attention-out tiles (all 4 projections)
QK_DT = F16            # dtype of Q/K tiles + mask-inject operands
PV_DT = F16            # dtype of V / A / A^T tiles (PV matmuls)
OUT_DT = F16           # DRAM output dtype (host upcasts to fp32)

_NP = {BF16: ml_dtypes.bfloat16, F16: np.float16, F32: np.float32}

LAST_RUN_WALL_S = None


def _drain_patch(self, tick_clock, wait_clock):
    """TileContext exit drain carries one sem-wait per instruction.

    The walrus in this container rejects a Drain with >1 sync wait
    ("Too many sync wait commands"), so split the global-clock waits
    onto single-wait SP nops before the drain."""
    nop0 = self.nc.sync.nop(nofuse=True)
    wait_clock.add_sem_waits(nop0.ins, ScopedClock({None: tick_clock.global_clock}))
    si = nop0.ins.sync_info
    waits = list(si.on_wait or []) if si is not None else []
    if len(waits) > 1:
        nop0.ins.sync_info = bass_rust.SyncInfo(
            on_wait=[waits[0]], on_update=list(si.on_update or [])
        )
        for w in waits[1:]:
            n2 = self.nc.sync.nop(nofuse=True)
            n2.ins.sync_info = bass_rust.SyncInfo(on_wait=[w], on_update=[])
    self.nc.sync.drain()
    self.nc.all_engine_barrier()
    popped = self.nc._tile_sem_poison_stack.pop()
    assert popped is self._sem_poison
    self.nc.clear_and_free_semaphores(list(self.sems.allocated().values()))
    self.nc.all_engine_barrier()


def _cast(ap, dt):
    return ap.bitcast(dt) if ap.dtype != dt else ap


_wait_split_installed = [False]


def _install_bir_wait_split():
    """The walrus in this container accepts at most ONE sync wait per
    instruction ("Too many sync wait commands").  Tile's scheduler freely
    emits several.  Rewrite the BIR JSON just before neuronxcc: any
    instruction with N>1 waits gets N-1 single-wait NoOps (same engine)
    inserted right before it — same semantics, engine program order
    preserved."""
    if _wait_split_installed[0]:
        return
    import json
    import concourse.bass2jax as b2j

    orig = b2j.compile_bir_kernel

    def patched(bir_json, tmpdir, neff_name="file.neff"):
        js = json.loads(bir_json)
        n_split = 0
        for fn in js.get("functions", []):
            for bb in fn.get("blocks", []):
                new_insts = []
                for inst in bb.get("instructions", []):
                    si = inst.get("sync_info")
                    ow = (si or {}).get("on_wait") or []
                    if len(ow) > 1:
                        for wi, w in enumerate(ow[:-1]):
                            new_insts.append({
                                "debug": inst.get("debug", 0),
                                "engine": inst["engine"],
                                "ins": [], "outs": [],
                                "name": f"{inst['name']}_wsplit{wi}",
                                "opcode": "NoOp",
                                "sync_info": {"on_update": [], "on_wait": [w]},
                            })
                            n_split += 1
                        si["on_wait"] = [ow[-1]]
                    new_insts.append(inst)
                bb["instructions"] = new_insts
        return orig(json.dumps(js).encode(), tmpdir, neff_name)

    b2j.compile_bir_kernel = patched
    _wait_split_installed[0] = True


PHASE_MARKS = []


def _mark(nc, label):
    PHASE_MARKS.append((label, nc.next_id()))


def build_program(with_biases, phases="all"):
    """One SPMD program; per-core differences come in through the inputs."""
    PHASE_MARKS.clear()
    nc = bass.Bass("TRN2", target_bir_lowering=False, debug=False,
                   num_devices=NCORES)
    AF = mybir.ActivationFunctionType

    pdt = PROJ_DT
    xT_d = nc.dram_tensor("xT", [D, EXT], pdt, kind="ExternalInput").ap()
    # Each core ships only ITS 128-row shard of every weight matrix; the
    # full matrices are rebuilt on-device with an AllGather over the 8
    # cores (NeuronLink), cutting host->device weight bytes 8x.
    wsh_d = {p: nc.dram_tensor(f"W{p}s", [128, D], pdt, kind="ExternalInput").ap()
             for p in "qkvo"}
    bqk_d = (nc.dram_tensor("bqk", [D, 2], F32, kind="ExternalInput").ap()
             if with_biases else None)
    bvo_d = (nc.dram_tensor("bvo", [2, D], F32, kind="ExternalInput").ap()
             if with_biases else None)
    m_d = [nc.dram_tensor(f"mask{i}", [QB, KW], QK_DT, kind="ExternalInput").ap()
           for i in range(3)]
    idqk_d = nc.dram_tensor("idqk", [128, 128], QK_DT, kind="ExternalInput").ap()
    idpv_d = (idqk_d if PV_DT == QK_DT else
              nc.dram_tensor("idpv", [128, 128], PV_DT, kind="ExternalInput").ap())
    # ONLY declare params that are actually read: an unused ExternalInput
    # gets dropped from the NEFF and the PJRT call then fails with an
    # input-count mismatch (INVALID_ARGUMENT).
    ones_d = (nc.dram_tensor("onesrow", [1, 128], PV_DT, kind="ExternalInput").ap()
              if with_biases else None)
    out_d = nc.dram_tensor("out", [INT, D], OUT_DT, kind="ExternalOutput").ap()

    with TileContext(nc) as tc:
        # All pools persist for the whole program: mid-context pool release
        # reuses memory without cross-pool synchronization (CoreSim flags
        # the race), so everything lives side by side instead.
        with tc.tile_pool(name="dram", bufs=1, space="DRAM") as dpool, \
             tc.tile_pool(name="const", bufs=1) as cpool, \
             tc.tile_pool(name="wpool", bufs=1) as wpool, \
             tc.tile_pool(name="qkpool", bufs=1) as qkpool, \
             tc.tile_pool(name="vpool", bufs=1) as vpool, \
             tc.tile_pool(name="xpool", bufs=1) as xpool, \
             tc.tile_pool(name="otpool", bufs=1) as otpool, \
             tc.tile_pool(name="apool", bufs=2) as apool, \
             tc.tile_pool(name="atpool", bufs=4) as atpool, \
             tc.tile_pool(name="smpool", bufs=3) as smpool, \
             tc.tile_pool(name="outpool", bufs=2) as outpool, \
             tc.tile_pool(name="ppsum", bufs=2, space="PSUM") as ppsum, \
             tc.tile_pool(name="spsum", bufs=2, space="PSUM") as spsum, \
             tc.tile_pool(name="atpsum", bufs=2, space="PSUM") as atpsum, \
             tc.tile_pool(name="pvpsum", bufs=2, space="PSUM") as pvpsum:

            # ---------------------------------------- weight all-gather
            # Issue the 4 collectives first, in consumption order, so the
            # wire time overlaps the x/mask/constant DMAs and each other.
            # The 4 gathers serialize on the collective datapath, so the
            # projections are emitted in the same q,k,v order and the
            # O-projection is deferred to the very end (Wo lands last).
            wgath = {}
            for p in "qkvo":
                wb = dpool.tile([128, D], pdt, tag=f"wb{p}", name=f"wb{p}")
                nc.gpsimd.dma_start(wb[:], wsh_d[p])
                gg = dpool.tile([NCORES, 128, D], pdt, tag=f"wg{p}",
                                name=f"wg{p}")
                nc.gpsimd.collective_compute(
                    "AllGather", mybir.AluOpType.bypass,
                    replica_groups=[list(range(NCORES))],
                    ins=[wb.opt()], outs=[gg.opt()])
                wgath[p] = gg

            # ------------------------------------------------ constants
            masks = []
            for i in range(3):
                mt = cpool.tile([QB, KW], QK_DT, tag=f"mask{i}", name=f"mask{i}_sb")
                nc.sync.dma_start(out=mt, in_=m_d[i])
                masks.append(mt)
            idqk = cpool.tile([128, 128], QK_DT, tag="idqk", name="idqk_sb")
            nc.sync.dma_start(out=idqk, in_=idqk_d)
            if PV_DT == QK_DT:
                idpv = idqk
            else:
                idpv = cpool.tile([128, 128], PV_DT, tag="idpv", name="idpv_sb")
                nc.sync.dma_start(out=idpv, in_=idpv_d)
            if with_biases:
                bqk = cpool.tile([128, 8, 2], F32, tag="bqk", name="bqk_sb")
                nc.sync.dma_start(
                    out=bqk, in_=bqk_d.rearrange("(m p) t -> p m t", p=128))
                bvo = cpool.tile([1, 2, D], PV_DT, tag="bvo", name="bvo_sb")
                nc.sync.dma_start(out=bvo, in_=bvo_d.rearrange("t d -> 1 t d"))
                onesrow = cpool.tile([1, 128], PV_DT, tag="ones", name="ones_sb")
                nc.sync.dma_start(out=onesrow, in_=ones_d)

            # ------------------------------------------------ persistent arrays
            QT = [qkpool.tile([128, INT], QK_DT, tag=f"qt{m}", name=f"qt{m}")
                  for m in range(8)]
            KT = [qkpool.tile([128, EXT], QK_DT, tag=f"kt{m}", name=f"kt{m}")
                  for m in range(8)]
            # V in natural layout, de-interleaved per parity; 4 full chunks
            # of 128 chain rows + one 16-row tail per parity
            VCH = [128, 128, 128, 128, 16]
            V = [[vpool.tile([VCH[v], D], PV_DT, tag=f"v{p}_{v}", name=f"v{p}_{v}")
                  for v in range(5)] for p in range(2)]
            OT = [otpool.tile([128, INT], pdt, tag=f"ot{m}", name=f"ot{m}")
                  for m in range(8)]

            xT = []
            for k in range(8):
                xt = xpool.tile([128, EXT], pdt, tag=f"xt{k}", name=f"xt{k}")
                nc.sync.dma_start(out=xt, in_=xT_d[k * 128:(k + 1) * 128, :])
                xT.append(xt)
            xTr = [t.rearrange("d (c two) -> d c two", two=2) for t in xT]

            # weight chunks stream through WSLOTS rotating single-buffer
            # slots so the next projection's chunks prefetch while the
            # current projection still holds its own
            wslot = [0]

            def load_w(which):
                tiles = []
                for k in range(8):
                    slot = (wslot[0] + k) % WSLOTS
                    wt = wpool.tile([128, D], pdt, tag=f"w{slot}",
                                    name=f"w_{which}{k}")
                    nc.sync.dma_start(out=wt, in_=wgath[which][k])
                    tiles.append(wt)
                wslot[0] = (wslot[0] + 8) % WSLOTS
                return tiles

            _mark(nc, 'setup_end')
            # ------------------------------------------------ projections
            # Q/K projections first (their gathers land first): out
            # (Q or K)^T [dout, seq]
            alt = [0]
            for which, dst, chunks, off, bcol in (
                    ("q", QT, [(0, 512), (512, 512)], HALO, 0),
                    ("k", KT, [(0, 512), (512, 512), (1024, 32)], 0, 1)):
                wt = load_w(which)
                for m in range(8):
                    for (s0, sl) in chunks:
                        ps = ppsum.tile([128, 512], F32, tag="ppsum", name="psQK")
                        for k in range(8):
                            nc.tensor.matmul(
                                ps[:, :sl],
                                lhsT=wt[k][:, m * 128:(m + 1) * 128],
                                rhs=xT[k][:, off + s0: off + s0 + sl],
                                start=(k == 0), stop=(k == 7))
                        if with_biases:
                            nc.scalar.activation(
                                dst[m][:, s0:s0 + sl], ps[:, :sl], AF.Identity,
                                bias=bqk[:, m, bcol:bcol + 1])
                        else:
                            alt[0] ^= 1
                            if alt[0]:
                                nc.scalar.copy(dst[m][:, s0:s0 + sl], ps[:, :sl])
                            else:
                                nc.vector.tensor_copy(dst[m][:, s0:s0 + sl],
                                                      ps[:, :sl])

            QTr = [t.rearrange("d (c two) -> d c two", two=2) for t in QT]
            KTr = [t.rearrange("d (c two) -> d c two", two=2) for t in KT]

            # -------------------------------- attention phase A (scores)
            # Emitted BEFORE the V projection: needs only Q/K, so the PE's
            # QK matmuls + ACT's exps + Pool/DVE's normalizes fill the
            # wait for the Wv gather (3rd in the collective chain).
            return_early = phases == "proj"
            ablocks = []
            for b in range(0 if return_early else NBLK):
                for p in range(2):
                    mt = masks[0] if b == 0 else (masks[2] if b == NBLK - 1 else masks[1])
                    sums = smpool.tile([128, 16], F32, tag="sums", name="sums")
                    A = apool.tile([128, 16, KW], PV_DT, tag=f"A{b}{p}",
                                   name=f"A{b}{p}")
                    # two heads per PSUM tile / Exp / rowsum-reduce: halves
                    # the per-instruction overhead on the ACT-serial chain
                    for j in range(8):
                        sps = spsum.tile([QB, 2, KW], F32, tag="s", name="spsum")
                        for i in range(2):
                            nc.tensor.matmul(sps[:, i, :], lhsT=idqk, rhs=mt,
                                             start=True, stop=False)
                            nc.tensor.matmul(
                                sps[:, i, :],
                                lhsT=QTr[j][i * 64:i * 64 + 64,
                                            b * QB:(b + 1) * QB, p],
                                rhs=KTr[j][i * 64:i * 64 + 64,
                                           b * QB:b * QB + KW, p],
                                start=False, stop=True)
                        nc.scalar.activation(
                            A[:, 2 * j:2 * j + 2, :], sps, AF.Exp, scale=0.125)
                        nc.vector.tensor_reduce(
                            sums[:, 2 * j:2 * j + 2], A[:, 2 * j:2 * j + 2, :],
                            axis=mybir.AxisListType.X, op=mybir.AluOpType.add)
                    rec = smpool.tile([128, 16], F32, tag="rec", name="rec")
                    nc.vector.reciprocal(rec, sums)
                    # normalize A in place; SBUF-only, so Pool can help DVE
                    for h in range(16):
                        if h % 2:
                            nc.gpsimd.tensor_scalar_mul(
                                A[:, h, :], A[:, h, :], rec[:, h:h + 1])
                        else:
                            nc.vector.tensor_scalar_mul(
                                A[:, h, :], A[:, h, :], rec[:, h:h + 1])
                    ablocks.append((b, p, A))

            _mark(nc, 'phaseA_end')
            # V projection: out V[p][v][rows, dout], lhsT = xT parity slice
            wv = load_w("v")
            for p in range(2):
                for v in range(5):
                    rows = VCH[v]
                    for n in range(2):
                        ps = ppsum.tile([128, 512], F32, tag="ppsum", name="psV")
                        for k in range(8):
                            nc.tensor.matmul(
                                ps[:rows, :],
                                lhsT=xTr[k][:, v * 128:v * 128 + rows, p],
                                rhs=wv[k][:, n * 512:(n + 1) * 512],
                                start=(k == 0), stop=(k == 7 and not with_biases))
                        if with_biases:
                            nc.tensor.matmul(
                                ps[:rows, :], lhsT=onesrow[:, :rows],
                                rhs=bvo[0:1, 0, n * 512:(n + 1) * 512],
                                start=False, stop=True)
                        eng = (v + n) % 2
                        if eng:
                            nc.scalar.copy(V[p][v][:rows, n * 512:(n + 1) * 512],
                                           ps[:rows, :])
                        else:
                            nc.vector.tensor_copy(V[p][v][:rows, n * 512:(n + 1) * 512],
                                                  ps[:rows, :])

            _mark(nc, 'vproj_end')
            wo = load_w("o")

            if return_early:
                # debug: skip attention/O-proj; dump V through out
                for s in range(8):
                    ot_out = outpool.tile([128, D], OUT_DT, tag="out", name="dbg_out")
                    nc.vector.tensor_copy(ot_out, V[s % 2][s % 4])
                    nc.sync.dma_start(out=out_d[s * 128:(s + 1) * 128, :], in_=ot_out)

            # ------------------------------- attention phase C (A^T @ V)
            # Two heads per iteration: one 512-wide psum tile holds both
            # heads' transposed A (main+halo), one merged copy moves it to
            # SBUF, and a 256-wide PV psum tile holds both heads' outputs.
            # Halves the number of chain round-trips per head.
            _mark(nc, 'preC_end')
            OTr = [t.rearrange("d (c two) -> d c two", two=2) for t in OT]
            for ci, (b, p, A) in enumerate(ablocks):
                for j in range(8):
                    h0 = 2 * j
                    atp = atpsum.tile([128, 512], PV_DT, tag="at", name="atpsum")
                    for i in range(2):
                        nc.tensor.transpose(atp[:, 256 * i:256 * i + 128],
                                            A[:, h0 + i, 0:QB], idpv)
                        nc.tensor.transpose(
                            atp[0:2 * KK, 256 * i + 128:256 * i + 256],
                            A[:, h0 + i, QB:KW], idpv)
                    at = atpool.tile([128, 512], PV_DT, tag="at", name="at_sb")
                    # merged copy (unwritten halo rows carry stale psum
                    # bits; PV never reads them), engine alternates per j
                    if (ci + j) % 2:
                        nc.scalar.copy(at, atp)
                    else:
                        nc.vector.tensor_copy(at, atp)
                    pvp = pvpsum.tile([64, 256], F32, tag="pv", name="pvpsum")
                    for i in range(2):
                        h = h0 + i
                        nc.tensor.matmul(
                            pvp[:, 128 * i:128 * i + 128],
                            lhsT=V[p][b][:, h * DH:(h + 1) * DH],
                            rhs=at[:, 256 * i:256 * i + 128],
                            start=True, stop=False)
                        nc.tensor.matmul(
                            pvp[:, 128 * i:128 * i + 128],
                            lhsT=V[p][b + 1][0:2 * KK, h * DH:(h + 1) * DH],
                            rhs=at[0:2 * KK, 256 * i + 128:256 * i + 256],
                            start=False, stop=True)
                    # the two heads share mch=j; dst rows differ (0:64 /
                    # 64:128) so the copies stay separate, one per engine
                    dst0 = OTr[j][0:64, b * QB:(b + 1) * QB, p]
                    dst1 = OTr[j][64:128, b * QB:(b + 1) * QB, p]
                    nc.vector.tensor_copy(dst0, pvp[:, 0:128])
                    nc.scalar.copy(dst1, pvp[:, 128:256])

            _mark(nc, 'phaseC_end')
            # ---------------------------------- output projection, deferred
            # until after all attention: Wo's gather lands last, and doing
            # any O-proj earlier head-of-line-blocks the in-order PE queue.
            for s in range(0 if return_early else 8):
                ot_out = outpool.tile([128, D], OUT_DT, tag="out", name="out_sb")
                for n in range(2):
                    ps = ppsum.tile([128, 512], F32, tag="ppsum", name="opsum")
                    for k in range(8):
                        nc.tensor.matmul(
                            ps,
                            lhsT=OT[k][:, s * 128:(s + 1) * 128],
                            rhs=wo[k][:, n * 512:(n + 1) * 512],
                            start=(k == 0), stop=(k == 7 and not with_biases))
                    if with_biases:
                        nc.tensor.matmul(
                            ps, lhsT=onesrow,
                            rhs=bvo[0:1, 1, n * 512:(n + 1) * 512],
                            start=False, stop=True)
                    if n:
                        nc.scalar.copy(ot_out[:, n * 512:(n + 1) * 512], ps)
                    else:
                        nc.vector.tensor_copy(ot_out[:, n * 512:(n + 1) * 512], ps)
                nc.sync.dma_start(out=out_d[s * 128:(s + 1) * 128, :], in_=ot_out)
    return nc


def _host_inputs(x, Wq, bq, Wk, bk, Wv, bv, Wo, bo):
    """Build the 8 per-core input maps."""
    qknp = _NP[QK_DT]
    pvnp = _NP[PV_DT]
    pnp = _NP[PROJ_DT]

    # band masks in block-local chain coords: allowed iff 0 <= j - i <= 16
    i = np.arange(QB)[:, None]
    j = np.arange(KW)[None, :]
    band = (j - i >= 0) & (j - i <= 2 * KK)
    m_mid = np.where(band, 0.0, NEG).astype(np.float32)
    # halo is KK chain positions wide; clip keys that fall outside [0, N)
    m_left = np.where(band & (j >= KK), 0.0, NEG).astype(np.float32)
    m_right = np.where(band & (j < KW - KK), 0.0, NEG).astype(np.float32)

    ident = np.eye(128, dtype=np.float32)
    bqk = np.stack([bq, bk], axis=1).astype(np.float32)          # [D, 2]
    bvo = np.stack([bv, bo], axis=0).astype(np.float32)          # [2, D]
    onesrow = np.ones((1, 128), dtype=np.float32)

    # pad + cast to the 16-bit wire dtype once, then slice per core
    xp = np.zeros((B, N + 2 * HALO, D), dtype=pnp)
    xp[:, HALO:HALO + N] = x

    w16 = {p: np.asarray(w, np.float32).astype(pnp)
           for p, w in (("q", Wq), ("k", Wk), ("v", Wv), ("o", Wo))}

    shared = {
        "bqk": bqk, "bvo": bvo.astype(pvnp),
        "mask1": m_mid.astype(qknp),
        "idqk": ident.astype(qknp),
        "onesrow": onesrow.astype(pvnp),
    }
    if PV_DT != QK_DT:
        shared["idpv"] = ident.astype(pvnp)

    with_biases = bool(np.any(bq) or np.any(bk) or np.any(bv) or np.any(bo))
    if not with_biases:
        shared.pop("bqk", None)
        shared.pop("bvo", None)
        shared.pop("onesrow", None)

    in_maps = []
    for core in range(NCORES):
        bi, half = core // 2, core % 2
        xT = np.ascontiguousarray(xp[bi, half * INT: half * INT + EXT].T)
        m0 = m_left if half == 0 else m_mid
        m2 = m_right if half == 1 else m_mid
        im = dict(shared)
        im["xT"] = xT
        # this core's 128-row shard of each weight matrix (AllGather
        # on-device rebuilds the full [1024, 1024])
        for p in "qkvo":
            im[f"W{p}s"] = np.ascontiguousarray(
                w16[p][core * 128:(core + 1) * 128])
        im["mask0"] = m0.astype(qknp)
        im["mask2"] = m2.astype(qknp)
        in_maps.append(im)
    return in_maps


_prog_cache = {}


def kernel(x, Wq, bq, Wk, bk, Wv, bv, Wo, bo, k, dilation, **_unused):
    x = np.asarray(x, np.float32)
    assert int(k) == KK and int(dilation) == DIL, (k, dilation)
    assert x.shape == (B, N, D)
    bq, bk, bv, bo = (np.asarray(v, np.float32).reshape(D) for v in (bq, bk, bv, bo))

    TileContext._drain_and_barrier = _drain_patch
    _install_bir_wait_split()

    with_biases = bool(np.any(bq) or np.any(bk) or np.any(bv) or np.any(bo))
    key = with_biases
    if key not in _prog_cache:
        _prog_cache[key] = build_program(with_biases)
    nc = _prog_cache[key]

    in_maps = _host_inputs(x, np.asarray(Wq), bq, np.asarray(Wk), bk,
                           np.asarray(Wv), bv, np.asarray(Wo), bo)
    import time as _time
    _t0 = _time.monotonic()
    res = run_bass_kernel_spmd(nc, in_maps, list(range(NCORES)))
    global LAST_RUN_WALL_S
    LAST_RUN_WALL_S = _time.monotonic() - _t0

    out = np.empty((B, N, D), dtype=np.float32)
    for core in range(NCORES):
        bi, half = core // 2, core % 2
        out[bi, half * INT:(half + 1) * INT] = res.results[core]["out"]  # f16 -> f32
    return out

